# revision 35
# baseline (speedup 1.0000x reference)
"""Deformable conv2d (3x3, pad 1) on 8 trn2 NeuronCores.

Sharding: (batch b, image half) -> core 2*b + half. Each core:
  0. PE p-state warmup (ldweights-free identity transposes) so the serial
     offset conv runs at full clock from its first matmul
  1. offset conv (PE matmuls over 2 c-tiles x 9 taps, N=400 position chunks)
  2. DMA-xbar transpose offsets to position-major; DVE index/bilinear math
     staged over two chunk ranges so early gathers overlap the prologue
  3. dma_gather of 2x2 fp16 pixel patches (all 256 ch) from an interleaved
     row-pair HBM image, positions-on-partitions
  4. bilinear combine, hybrid per tap: taps < N_DIAG run diag-matmul
     accumulation on PE (rhs = batched broadcast-built scaled identities);
     the rest run per-partition beta scales + corner sums on DVE (fp16 4x)
     with plain PE transposes (fixed identity stationary)
  5. main conv = 18-chunk PE accumulation over (c-tile, tap); conv bias is
     applied by the output activation copy (per-partition bias), fp16 store.
"""
import numpy as np

B, CIN, COUT, H, W = 4, 256, 256, 80, 80
NCORES = 8
HHALF = H // 2                      # 40 rows per core
NPOS = HHALF * W                    # 3200 positions per core
NCHUNK = NPOS // 128                # 25
PITCH = 84                          # x2 pixel-group pitch per row
NGROUPS = PITCH * PITCH             # 7056 (rows in x2; 83*84 + slack)
FBIAS = 16.0                        # float->int truncation bias (floor trick)
CLIP_LO = 14.0                      # = -2 + FBIAS
CLIP_HI = 96.9                      # = 80.9 + FBIAS
FLAT_OFF = -(14 * PITCH + 14)       # flat = y0b*84 + x0b + FLAT_OFF
GROUPS = [(0, 4), (4, 4), (8, 4), (12, 4), (16, 4), (20, 4), (24, 1)]
N_DIAG = 9          # all taps use the PE diag-matmul bilinear path

_cached = {}


def _build_program():
    from concourse import bass, bacc, tile, mybir
    from contextlib import ExitStack

    fp16, fp32 = mybir.dt.float16, mybir.dt.float32
    i16, i32 = mybir.dt.int16, mybir.dt.int32
    A = mybir.AluOpType
    ACT_COPY = mybir.ActivationFunctionType.Copy
    ACT_IDENT = mybir.ActivationFunctionType.Identity

    nc = bacc.Bacc("TRN2", target_bir_lowering=False, debug=False,
                   num_devices=NCORES, num_swdge_queues=4)

    x2_d = nc.dram_tensor("x2", [NGROUPS, 512], fp16, kind="ExternalInput")
    xcf_d = nc.dram_tensor("xcf", [128, 2, 44 * PITCH], fp16, kind="ExternalInput")
    offw_d = nc.dram_tensor("offw", [128, 2, 9, 18], fp16, kind="ExternalInput")
    offb_d = nc.dram_tensor("offb", [18, 1], fp32, kind="ExternalInput")
    convw_d = nc.dram_tensor("convw", [128, 2, 9, 256], fp16, kind="ExternalInput")
    convb_d = nc.dram_tensor("convb", [128, 2], fp32, kind="ExternalInput")
    cyb_d = nc.dram_tensor("cyb", [128, NCHUNK, 9], fp32, kind="ExternalInput")
    cxb_d = nc.dram_tensor("cxb", [128, NCHUNK, 9], fp32, kind="ExternalInput")
    id16_d = nc.dram_tensor("id16", [128, 128], fp16, kind="ExternalInput")
    sel_d = nc.dram_tensor("sel", [128, 8, 2, 128], fp16, kind="ExternalInput")
    out_d = nc.dram_tensor("out", [2, 128, NPOS], fp16, kind="ExternalOutput")

    # overlapping gather-source view: [NGROUPS-1, 1024] with row stride 512
    x2_view = x2_d.ap().copy()
    v = x2_view.ap
    v[0] = [512, NGROUPS - 1]
    v[1] = [1, 1024]
    x2_view.ap = v

    def revec(ap, dims, extra_offset=0):
        """Rebuild an AP's dim list: dims = [(stride, num), ...]."""
        a = ap.copy()
        vv = a.ap
        while len(vv) > 1:
            vv.pop()
        vv[0] = list(dims[0])
        for d in dims[1:]:
            vv.append(list(d))
        a.ap = vv
        a.offset = a.offset + extra_offset
        return a

    with tile.TileContext(nc) as tc:
        with ExitStack() as ctx:
            persist = ctx.enter_context(tc.tile_pool(name="persist", bufs=1))
            idxw = persist.tile([128, 9, NPOS // 16], i16)
            beta = persist.tile([128, NCHUNK, 9, 4], fp32)
            id16 = persist.tile([128, 128], fp16)
            convw = persist.tile([128, 2, 9, 256], fp16)
            convb = persist.tile([128, 2], fp32)
            # gather buffers live in the persist pool so their addresses never
            # overlap prologue scratch: otherwise the first gathers WAR-wait
            # on the last index-stage's reads (~60us)
            gbt = {}
            for k in range(9):
                gbt[k] = persist.tile([128, (2 if k >= 6 else 1), 4, 1024],
                                      fp16, name=f"gbt{k}", tag=f"gb{k}")
            # ---------------- prologue: offsets + indices ----------------
            with ExitStack() as pctx:
                ppool = pctx.enter_context(tc.tile_pool(name="pro", bufs=1))
                ppsum = pctx.enter_context(
                    tc.tile_pool(name="ppsum", bufs=2, space="PSUM"))

                xcf = ppool.tile([128, 2, 44 * PITCH], fp16)
                cyb = ppool.tile([128, NCHUNK, 9], fp32)
                cxb = ppool.tile([128, NCHUNK, 9], fp32)
                off_sb = ppool.tile([32, NPOS], fp16)
                offT = ppool.tile([128, NCHUNK, 32], fp16)
                nc.vector.memset(off_sb[:], 0.0)
                offw = ppool.tile([128, 2, 9, 18], fp16)
                offb = ppool.tile([18, 1], fp32)
                sel = ppool.tile([128, 8, 2, 128], fp16)
                nc.sync.dma_start(out=sel[:], in_=sel_d[:])
                nc.sync.dma_start(out=id16[:], in_=id16_d[:])
                nc.sync.dma_start(out=offw[:], in_=offw_d[:])
                nc.sync.dma_start(out=offb[:], in_=offb_d[:])
                SPL1, SPL2 = 8 * PITCH, 20 * PITCH
                nc.sync.dma_start(out=xcf[:, :, :SPL1], in_=xcf_d[:, :, :SPL1])
                nc.sync.dma_start(out=xcf[:, :, SPL1:SPL2],
                                  in_=xcf_d[:, :, SPL1:SPL2])
                nc.sync.dma_start(out=xcf[:, :, SPL2:], in_=xcf_d[:, :, SPL2:])
                nc.sync.dma_start(out=cyb[:], in_=cyb_d[:])
                nc.sync.dma_start(out=cxb[:], in_=cxb_d[:])
                nc.sync.dma_start(out=convw[:], in_=convw_d[:])
                nc.sync.dma_start(out=convb[:], in_=convb_d[:])

                # PE p-state warmup: ldweights-free transposes keep the PE
                # continuously busy from t~0 so the offset conv starts at
                # full clock instead of ramping from the low p-state.
                wps = ppsum.tile([128, 128], fp16, tag="warm", bufs=1)
                for _ in range(45):
                    nc.tensor.matmul(wps[:], lhsT=id16[:], rhs=id16[:],
                                     start=True, stop=True, is_transpose=True)

                # offset conv: 8 chunks x 5 rows x 80 cols (N=400).
                # xcf holds the core's 44-row padded window relocated to rows
                # [0..44); conv output row r (0..39) reads xcf rows (r + ky).
                def conv_chunk(rc):
                    ps = ppsum.tile([18, 400], fp32, tag="offps")
                    mm = 0
                    for ct in range(2):
                        for t in range(9):
                            ky, kx = t // 3, t % 3
                            base = (rc * 5 + ky + 1) * PITCH + kx + 1
                            mov = revec(xcf[:, ct, 0],
                                        [(2 * 44 * PITCH, 128), (PITCH, 5), (1, 80)],
                                        extra_offset=base)
                            nc.tensor.matmul(
                                ps[:], lhsT=offw[:, ct, t, :], rhs=mov,
                                start=(mm == 0), stop=(mm == 17))
                            mm += 1
                    nc.scalar.activation(off_sb[0:18, rc * 400:(rc + 1) * 400], ps[:],
                                         ACT_IDENT, bias=offb[:])

                # index pipeline in two chunk-range stages so early-block
                # gathers start while the rest of the prologue still runs
                def index_stage(c0, c1):
                    nch = c1 - c0
                    ne = nch * 9
                    # transpose offsets via DMA xbar: [18, nch*128] -> [128, nch, 18]
                    nc.scalar.dma_start_transpose(
                        offT[:, c0:c1, :],
                        off_sb[:, c0 * 128:c1 * 128])

                    dy = revec(offT[:], [(NCHUNK * 32, 128), (32, nch), (2, 9)],
                               c0 * 32)
                    dx = revec(offT[:], [(NCHUNK * 32, 128), (32, nch), (2, 9)],
                               c0 * 32 + 1)
                    cybv = cyb[:, c0:c1, :]
                    cxbv = cxb[:, c0:c1, :]
                    pyb = ppool.tile([128, ne], fp32, tag="t_pyb")
                    pxb = ppool.tile([128, ne], fp32, tag="t_pxb")
                    t_i32 = ppool.tile([128, ne], i32, tag="t_ti")
                    y0f = ppool.tile([128, ne], fp32, tag="t_y0")
                    x0f = ppool.tile([128, ne], fp32, tag="t_x0")
                    fy = ppool.tile([128, ne], fp32, tag="t_fy")
                    fx = ppool.tile([128, ne], fp32, tag="t_fx")
                    gy = ppool.tile([128, ne], fp32, tag="t_gy")
                    gx = ppool.tile([128, ne], fp32, tag="t_gx")
                    gtt = ppool.tile([128, ne], fp32, tag="t_gt")
                    hi16 = ppool.tile([128, ne], fp16, tag="t_hi")
                    lo16 = ppool.tile([128, ne], fp16, tag="t_lo")

                    v3 = lambda t: revec(t[:], [(ne, 128), (9, nch), (1, 9)])
                    nc.vector.tensor_tensor(out=pyb[:], in0=dy, in1=cybv, op=A.add)
                    nc.vector.tensor_tensor(out=pxb[:], in0=dx, in1=cxbv, op=A.add)
                    nc.vector.tensor_scalar(pyb[:], pyb[:], CLIP_LO, CLIP_HI,
                                            A.max, A.min)
                    nc.vector.tensor_scalar(pxb[:], pxb[:], CLIP_LO, CLIP_HI,
                                            A.max, A.min)
                    # robust floor (works under trunc or round-to-nearest cvt):
                    # y0 = cvt(pyb); y0 -= (y0 > pyb)
                    nc.vector.tensor_copy(t_i32[:], pyb[:])
                    nc.vector.tensor_copy(y0f[:], t_i32[:])
                    nc.vector.tensor_tensor(out=gtt[:], in0=y0f[:], in1=pyb[:],
                                            op=A.is_gt)
                    nc.vector.tensor_tensor(out=y0f[:], in0=y0f[:], in1=gtt[:],
                                            op=A.subtract)
                    nc.vector.tensor_copy(t_i32[:], pxb[:])
                    nc.vector.tensor_copy(x0f[:], t_i32[:])
                    nc.vector.tensor_tensor(out=gtt[:], in0=x0f[:], in1=pxb[:],
                                            op=A.is_gt)
                    nc.vector.tensor_tensor(out=x0f[:], in0=x0f[:], in1=gtt[:],
                                            op=A.subtract)
                    nc.vector.tensor_tensor(out=fy[:], in0=pyb[:], in1=y0f[:],
                                            op=A.subtract)
                    nc.vector.tensor_tensor(out=fx[:], in0=pxb[:], in1=x0f[:],
                                            op=A.subtract)
                    # flat = (y0b*84 + x0b) - 1190, via stt then scalar add
                    nc.vector.scalar_tensor_tensor(pyb[:], y0f[:], float(PITCH),
                                                   x0f[:], A.mult, A.add)
                    nc.vector.tensor_scalar_add(pyb[:], pyb[:], float(FLAT_OFF))
                    # gy = 1-fy, gx = 1-fx
                    nc.vector.tensor_scalar(gy[:], fy[:], -1.0, 1.0, A.mult, A.add)
                    nc.vector.tensor_scalar(gx[:], fx[:], -1.0, 1.0, A.mult, A.add)
                    # beta[j]: b0=gx*gy b1=gx*fy b2=fx*gy b3=fx*fy
                    bj = lambda j: revec(beta[:],
                                         [(NE * 4, 128), (36, nch), (4, 9)],
                                         j + c0 * 36)
                    nc.vector.tensor_tensor(out=bj(0), in0=v3(gx), in1=v3(gy),
                                            op=A.mult)
                    nc.vector.tensor_tensor(out=bj(1), in0=v3(gx), in1=v3(fy),
                                            op=A.mult)
                    nc.vector.tensor_tensor(out=bj(2), in0=v3(fx), in1=v3(gy),
                                            op=A.mult)
                    nc.vector.tensor_tensor(out=bj(3), in0=v3(fx), in1=v3(fy),
                                            op=A.mult)

                    # idx fold via PE: flat (fp32 in pyb, values [0, 7056))
                    # split flat = 64*hi + lo (both fp16-exact), then per
                    # source group g two accumulating matmuls with selection
                    # matrices (entries 64.0 / 1.0) produce, replicated on
                    # every 16-partition stripe, out[16g'+q, (c,k)] =
                    # flat[16g+q, c, k]; strided casts land idxw[:, k, 8c+g].
                    # robust floor for hi (cvt may trunc or round-to-nearest)
                    nc.vector.tensor_scalar_mul(x0f[:], pyb[:], 1.0 / 64.0)
                    nc.vector.tensor_copy(t_i32[:], x0f[:])
                    nc.vector.tensor_copy(y0f[:], t_i32[:])
                    nc.vector.tensor_tensor(out=gtt[:], in0=y0f[:], in1=x0f[:],
                                            op=A.is_gt)
                    nc.vector.tensor_tensor(out=y0f[:], in0=y0f[:], in1=gtt[:],
                                            op=A.subtract)
                    nc.vector.tensor_copy(hi16[:], y0f[:])
                    nc.vector.scalar_tensor_tensor(x0f[:], y0f[:], -64.0,
                                                   pyb[:], A.mult, A.add)
                    nc.vector.tensor_copy(lo16[:], x0f[:])
                    for g in range(8):
                        fps = ppsum.tile([128, ne], fp32, tag="foldps", bufs=2)
                        nc.tensor.matmul(fps[:], lhsT=sel[:, g, 0, :],
                                         rhs=hi16[:], start=True, stop=False)
                        nc.tensor.matmul(fps[:], lhsT=sel[:, g, 1, :],
                                         rhs=lo16[:], start=False, stop=True)
                        f32 = ppool.tile([128, ne], i32, tag=f"t_f32_{g % 2}")
                        nc.vector.tensor_copy(f32[:], fps[:])
                        src = revec(f32[:], [(ne, 128), (1, 9), (9, nch)])
                        dst = revec(idxw[:], [(9 * (NPOS // 16), 128),
                                              (NPOS // 16, 9), (8, nch)],
                                    extra_offset=c0 * 8 + g)
                        nc.vector.tensor_copy(dst, src)

                NE = NCHUNK * 9   # 225
                # 4-stage pipeline: each pair of conv chunks (800 positions)
                # unlocks the next 6-7 index chunks, so the first gathers
                # launch while most of the offset conv is still running
                conv_chunk(0)
                conv_chunk(1)
                index_stage(0, 6)
                conv_chunk(2)
                conv_chunk(3)
                index_stage(6, 12)
                conv_chunk(4)
                conv_chunk(5)
                index_stage(12, 18)
                conv_chunk(6)
                conv_chunk(7)
                index_stage(18, NCHUNK)

            # ---------------- main loop ----------------
            with ExitStack() as mctx:
                mpool = mctx.enter_context(tc.tile_pool(name="main", bufs=1))
                mpsum = mctx.enter_context(
                    tc.tile_pool(name="mpsum", bufs=1, space="PSUM"))

                for gidx, (c0g, ng) in enumerate(GROUPS):
                    npos = ng * 128
                    base = c0g * 128
                    gbi = [gidx % (2 if k >= 6 else 1) for k in range(9)]
                    for k in range(9):
                        nc.gpsimd.dma_gather(
                            gbt[k][:, gbi[k], :ng, :], x2_view,
                            idxw[:, k, c0g * 8:(c0g + ng) * 8],
                            npos, npos, 1024, elem_step=512,
                            queue_num=k % 4)

                    valbuf = mpool.tile([128, 18, 512], fp16, tag="valbuf", bufs=2)

                    # batched diag-matrix build for the A-path taps:
                    # dt4[i][p, kk, j, c] = id16[p, c] * beta[p, c0g+i, kk, j]
                    dt4s = []
                    for i in range(ng):
                        ch = c0g + i
                        dt4 = mpool.tile([128, N_DIAG, 4, 128], fp16,
                                         tag="dt4", bufs=4)
                        in0 = revec(id16[:],
                                    [(128, 128), (0, N_DIAG * 4), (1, 128)])
                        in1 = revec(beta[:],
                                    [(NCHUNK * 36, 128), (1, N_DIAG * 4),
                                     (0, 128)], ch * 36)
                        nc.vector.tensor_tensor(out=dt4[:], in0=in0, in1=in1,
                                                op=A.mult)
                        dt4s.append(dt4)

                    for k in range(9):
                        if k < N_DIAG:
                            # PE diag-matmul path (scale+sum+transpose on PE)
                            pv0 = mpsum.tile([128, 512], fp32, tag="pva0",
                                             bufs=2)
                            pv1 = mpsum.tile([128, 512], fp32, tag="pva1",
                                             bufs=2)
                            for i in range(ng):
                                for ct, pv in ((0, pv0), (1, pv1)):
                                    for j in range(4):
                                        slot = (j // 2) * 4 + (j % 2) * 2 + ct
                                        nc.tensor.matmul(
                                            pv[:, i * 128:(i + 1) * 128],
                                            lhsT=gbt[k][:, gbi[k], i,
                                                        slot * 128:(slot + 1) * 128],
                                            rhs=dt4s[i][:, k, j, :],
                                            start=(j == 0), stop=(j == 3))
                        else:
                            # scale+sum path (scalar j0/j1, DVE j2/j3 fused)
                            # with plain PE transposes
                            pv0 = mpsum.tile([128, 512], fp16, tag="pvf0",
                                             bufs=2)
                            pv1 = mpsum.tile([128, 512], fp16, tag="pvf1",
                                             bufs=2)
                            for i in range(ng):
                                ch = c0g + i
                                scr = mpool.tile([128, 4, 256], fp16,
                                                 tag="scr", bufs=4)
                                s2 = mpool.tile([128, 2, 256], fp16,
                                                tag="s2", bufs=4)
                                summ = mpool.tile([128, 256], fp16,
                                                  tag="summ", bufs=4)
                                for j in range(2):
                                    nc.scalar.activation(
                                        scr[:, j, :],
                                        gbt[k][:, gbi[k], i, j * 256:(j + 1) * 256],
                                        ACT_COPY,
                                        scale=beta[:, ch, k, j:j + 1])
                                bb = revec(beta[:],
                                           [(NCHUNK * 36, 128), (1, 2),
                                            (0, 256)], ch * 36 + k * 4 + 2)
                                nc.vector.tensor_tensor(
                                    out=scr[:, 2:4, :],
                                    in0=gbt[k][:, gbi[k], i, 512:1024],
                                    in1=bb, op=A.mult)
                                nc.vector.tensor_tensor(
                                    out=s2[:], in0=scr[:, 0:2, :],
                                    in1=scr[:, 2:4, :], op=A.add)
                                nc.vector.tensor_tensor(
                                    out=summ[:], in0=s2[:, 0, :],
                                    in1=s2[:, 1, :], op=A.add)
                                nc.tensor.matmul(
                                    pv0[:, i * 128:(i + 1) * 128],
                                    lhsT=summ[:, 0:128],
                                    rhs=id16[:], start=True, stop=True,
                                    is_transpose=True)
                                nc.tensor.matmul(
                                    pv1[:, i * 128:(i + 1) * 128],
                                    lhsT=summ[:, 128:256],
                                    rhs=id16[:], start=True, stop=True,
                                    is_transpose=True)
                        nc.scalar.activation(valbuf[:, k, :npos],
                                             pv0[:, :npos], ACT_COPY)
                        nc.scalar.activation(valbuf[:, 9 + k, :npos],
                                             pv1[:, :npos], ACT_COPY)

                    for ot in range(2):
                        po = mpsum.tile([128, 512], fp32, tag=f"po{ot}", bufs=1)
                        for ci in range(18):
                            ct, k = ci // 9, ci % 9
                            nc.tensor.matmul(
                                po[:, :npos],
                                lhsT=convw[:, ct, k, ot * 128:(ot + 1) * 128],
                                rhs=valbuf[:, ci, :npos],
                                start=(ci == 0), stop=(ci == 17))
                        osb = mpool.tile([128, 512], fp16, tag="osb", bufs=2)
                        nc.scalar.activation(osb[:, :npos], po[:, :npos],
                                             ACT_IDENT, bias=convb[:, ot:ot + 1])
                        nc.sync.dma_start(
                            out=out_d[ot, :, base:base + npos],
                            in_=osb[:, :npos])

    nc.compile()
    return nc


def _host_prep(x, offset_w, offset_b, conv_w, conv_b):
    """Build per-core input maps."""
    x = np.asarray(x, np.float32)
    offset_w = np.asarray(offset_w, np.float32)
    offset_b = np.asarray(offset_b, np.float32)
    conv_w = np.asarray(conv_w, np.float32)
    conv_b = np.asarray(conv_b, np.float32)

    # weights, shared
    # offset_w: [18, 256, 3, 3] -> [c128, ct, t, d]
    ow = offset_w.reshape(18, 2, 128, 3, 3)
    offw_h = np.ascontiguousarray(
        ow.reshape(18, 2, 128, 9).transpose(2, 1, 3, 0)).astype(np.float16)
    offb_h = offset_b.reshape(18, 1).astype(np.float32)
    cw = conv_w.reshape(256, 2, 128, 9)
    convw_h = np.ascontiguousarray(cw.transpose(2, 1, 3, 0)).astype(np.float16)  # [c,ct,t,o]
    convb_h = np.ascontiguousarray(
        conv_b.reshape(2, 128).transpose(1, 0)).astype(np.float32)  # [c128, ot]
    id16_h = np.eye(128, dtype=np.float16)
    # fold selection matrices: sel[r, g, 0/1, m] = 64/1 at m%16 == r%16 for
    # source group g == r//16, replicated over all 8 dest 16-part stripes
    sel_h = np.zeros((128, 8, 2, 128), np.float16)
    r = np.arange(128)
    m = np.arange(128)
    hit = (m[None, :] % 16) == (r[:, None] % 16)      # [r, m]
    for g in range(8):
        gm = hit & ((r[:, None] // 16) == g)
        sel_h[:, g, 0, :] = gm * np.float16(64.0)
        sel_h[:, g, 1, :] = gm * np.float16(1.0)

    # per-core base constants
    k = np.arange(9)
    ry = (k // 3 - 1).astype(np.float32)
    rx = (k % 3 - 1).astype(np.float32)
    in_maps = []
    per_sample = {}
    for b in range(B):
        xc = np.ascontiguousarray(x[b].transpose(1, 2, 0))       # [H, W, C]
        xp = np.pad(xc, ((2, 2), (2, 2), (0, 0))).astype(np.float16)  # [84, 84, 256]
        x2 = np.zeros((PITCH, PITCH, 2, 256), np.float16)
        x2[:83, :, 0] = xp[:83]
        x2[:83, :, 1] = xp[1:84]
        x2_h = x2.reshape(NGROUPS, 512)
        per_sample[b] = (x2_h, xp)

    for core in range(NCORES):
        b, half = core // 2, core % 2
        h0 = half * HHALF
        x2_h, xp = per_sample[b]
        # xcf: channel-first, rows [h0-2 .. h0+42) of the padded image
        # relocated to local rows [0..44)
        xcf_rows = xp[h0:h0 + 44]                                # [44, 84, 256]
        xcf_h = np.ascontiguousarray(
            xcf_rows.transpose(2, 0, 1).reshape(2, 128, 44 * PITCH)
            .transpose(1, 0, 2))

        i = np.arange(NPOS)
        hloc = i // W
        wloc = i % W
        cyb_h = ((h0 + hloc)[:, None] + ry[None, :] + FBIAS).astype(np.float32)
        cxb_h = (wloc[:, None] + rx[None, :] + FBIAS).astype(np.float32)
        cyb_h = np.ascontiguousarray(
            cyb_h.reshape(NCHUNK, 128, 9).transpose(1, 0, 2))
        cxb_h = np.ascontiguousarray(
            cxb_h.reshape(NCHUNK, 128, 9).transpose(1, 0, 2))

        in_maps.append({
            "x2": x2_h, "xcf": xcf_h, "offw": offw_h, "offb": offb_h,
            "convw": convw_h, "convb": convb_h, "cyb": cyb_h, "cxb": cxb_h,
            "id16": id16_h, "sel": sel_h,
        })
    return in_maps


def kernel(x, offset_w, offset_b, conv_w, conv_b, _trace=False):
    from concourse.bass_utils import run_bass_kernel_spmd

    if "nc" not in _cached:
        _cached["nc"] = _build_program()
    nc = _cached["nc"]
    in_maps = _host_prep(x, offset_w, offset_b, conv_w, conv_b)
    res = run_bass_kernel_spmd(nc, in_maps, list(range(NCORES)), trace=_trace)
    _cached["last_result"] = res
    out = np.zeros((B, COUT, H, W), np.float32)
    for core in range(NCORES):
        b, half = core // 2, core % 2
        o = res.results[core]["out"].astype(np.float32)   # [2, 128, NPOS]
        out[b, :, half * HHALF:(half + 1) * HHALF, :] = \
            o.reshape(COUT, HHALF, W)
    return out



# revision 38
# speedup vs baseline: 1.2678x; 1.2678x over previous
"""Deformable conv2d (3x3, pad 1) on 8 trn2 NeuronCores.

Sharding: (batch b, image half) -> core 2*b + half. Each core:
  0. PE p-state warmup (ldweights-free identity transposes) so the serial
     offset conv runs at full clock from its first matmul
  1. offset conv (PE matmuls over 2 c-tiles x 9 taps, N=400 position chunks)
  2. DMA-xbar transpose offsets to position-major; DVE index/bilinear math
     staged over two chunk ranges so early gathers overlap the prologue
  3. dma_gather of 2x2 fp16 pixel patches (all 256 ch) from an interleaved
     row-pair HBM image, positions-on-partitions
  4. bilinear combine, hybrid per tap: taps < N_DIAG run diag-matmul
     accumulation on PE (rhs = batched broadcast-built scaled identities);
     the rest run per-partition beta scales + corner sums on DVE (fp16 4x)
     with plain PE transposes (fixed identity stationary)
  5. main conv = 18-chunk PE accumulation over (c-tile, tap); conv bias is
     applied by the output activation copy (per-partition bias), fp16 store.
"""
import numpy as np

B, CIN, COUT, H, W = 4, 256, 256, 80, 80
NCORES = 8
HHALF = H // 2                      # 40 rows per core
NPOS = HHALF * W                    # 3200 positions per core
NCHUNK = NPOS // 128                # 25
PITCH = 84                          # x2 pixel-group pitch per row
NGROUPS = PITCH * PITCH             # 7056 (rows in x2; 83*84 + slack)
FBIAS = 16.0                        # float->int truncation bias (floor trick)
CLIP_LO = 14.0                      # = -2 + FBIAS
CLIP_HI = 96.9                      # = 80.9 + FBIAS
FLAT_OFF = -(14 * PITCH + 14)       # flat = y0b*84 + x0b + FLAT_OFF
GROUPS = [(0, 4), (4, 4), (8, 4), (12, 4), (16, 4), (20, 4), (24, 1)]
N_DIAG = 9          # all taps use the PE diag-matmul bilinear path

_cached = {}


def _build_program():
    from concourse import bass, bacc, tile, mybir
    from contextlib import ExitStack

    fp16, fp32 = mybir.dt.float16, mybir.dt.float32
    i16, i32 = mybir.dt.int16, mybir.dt.int32
    A = mybir.AluOpType
    ACT_COPY = mybir.ActivationFunctionType.Copy
    ACT_IDENT = mybir.ActivationFunctionType.Identity

    nc = bacc.Bacc("TRN2", target_bir_lowering=False, debug=False,
                   num_devices=NCORES, num_swdge_queues=4)

    x2_d = nc.dram_tensor("x2", [NGROUPS, 512], fp16, kind="ExternalInput")
    xcf_d = nc.dram_tensor("xcf", [128, 2, 44 * PITCH], fp16, kind="ExternalInput")
    offw_d = nc.dram_tensor("offw", [128, 2, 9, 18], fp16, kind="ExternalInput")
    offb_d = nc.dram_tensor("offb", [18, 1], fp32, kind="ExternalInput")
    convw_d = nc.dram_tensor("convw", [128, 2, 9, 256], fp16, kind="ExternalInput")
    convb_d = nc.dram_tensor("convb", [128, 2], fp32, kind="ExternalInput")
    cyb_d = nc.dram_tensor("cyb", [128, NCHUNK, 9], fp32, kind="ExternalInput")
    cxb_d = nc.dram_tensor("cxb", [128, NCHUNK, 9], fp32, kind="ExternalInput")
    id16_d = nc.dram_tensor("id16", [128, 128], fp16, kind="ExternalInput")
    sel_d = nc.dram_tensor("sel", [128, 8, 2, 128], fp16, kind="ExternalInput")
    out_d = nc.dram_tensor("out", [2, 128, NPOS], fp16, kind="ExternalOutput")

    # overlapping gather-source view: [NGROUPS-1, 1024] with row stride 512
    x2_view = x2_d.ap().copy()
    v = x2_view.ap
    v[0] = [512, NGROUPS - 1]
    v[1] = [1, 1024]
    x2_view.ap = v

    def revec(ap, dims, extra_offset=0):
        """Rebuild an AP's dim list: dims = [(stride, num), ...]."""
        a = ap.copy()
        vv = a.ap
        while len(vv) > 1:
            vv.pop()
        vv[0] = list(dims[0])
        for d in dims[1:]:
            vv.append(list(d))
        a.ap = vv
        a.offset = a.offset + extra_offset
        return a

    with tile.TileContext(nc) as tc:
        with ExitStack() as ctx:
            persist = ctx.enter_context(tc.tile_pool(name="persist", bufs=1))
            idxw = persist.tile([128, 9, NPOS // 16], i16)
            beta = persist.tile([128, NCHUNK, 9, 4], fp16)
            id16 = persist.tile([128, 128], fp16)
            convw = persist.tile([128, 2, 9, 256], fp16)
            convb = persist.tile([128, 2], fp32)

            # ---------------- prologue: offsets + indices ----------------
            with ExitStack() as pctx:
                ppool = pctx.enter_context(tc.tile_pool(name="pro", bufs=1))
                ppsum = pctx.enter_context(
                    tc.tile_pool(name="ppsum", bufs=2, space="PSUM"))

                xcf = ppool.tile([128, 2, 44 * PITCH], fp16)
                cyb = ppool.tile([128, NCHUNK, 9], fp32)
                cxb = ppool.tile([128, NCHUNK, 9], fp32)
                off_sb = ppool.tile([32, NPOS], fp16)
                offT = ppool.tile([128, NCHUNK, 32], fp16)
                nc.vector.memset(off_sb[:], 0.0)
                offw = ppool.tile([128, 2, 9, 18], fp16)
                offb = ppool.tile([18, 1], fp32)
                sel = ppool.tile([128, 8, 2, 128], fp16)
                nc.sync.dma_start(out=sel[:], in_=sel_d[:])
                nc.sync.dma_start(out=id16[:], in_=id16_d[:])
                nc.sync.dma_start(out=offw[:], in_=offw_d[:])
                nc.sync.dma_start(out=offb[:], in_=offb_d[:])
                SPL1, SPL2 = 8 * PITCH, 20 * PITCH
                nc.sync.dma_start(out=xcf[:, :, :SPL1], in_=xcf_d[:, :, :SPL1])
                nc.sync.dma_start(out=xcf[:, :, SPL1:SPL2],
                                  in_=xcf_d[:, :, SPL1:SPL2])
                nc.sync.dma_start(out=xcf[:, :, SPL2:], in_=xcf_d[:, :, SPL2:])
                nc.sync.dma_start(out=cyb[:], in_=cyb_d[:])
                nc.sync.dma_start(out=cxb[:], in_=cxb_d[:])
                nc.sync.dma_start(out=convw[:], in_=convw_d[:])
                nc.sync.dma_start(out=convb[:], in_=convb_d[:])

                # PE p-state warmup: ldweights-free transposes keep the PE
                # continuously busy from t~0 so the offset conv starts at
                # full clock instead of ramping from the low p-state.
                wps = ppsum.tile([128, 128], fp16, tag="warm", bufs=1)
                for _ in range(45):
                    nc.tensor.matmul(wps[:], lhsT=id16[:], rhs=id16[:],
                                     start=True, stop=True, is_transpose=True)

                # offset conv: 8 chunks x 5 rows x 80 cols (N=400).
                # xcf holds the core's 44-row padded window relocated to rows
                # [0..44); conv output row r (0..39) reads xcf rows (r + ky).
                def conv_chunk(rc):
                    ps = ppsum.tile([18, 400], fp32, tag="offps")
                    mm = 0
                    for ct in range(2):
                        for t in range(9):
                            ky, kx = t // 3, t % 3
                            base = (rc * 5 + ky + 1) * PITCH + kx + 1
                            mov = revec(xcf[:, ct, 0],
                                        [(2 * 44 * PITCH, 128), (PITCH, 5), (1, 80)],
                                        extra_offset=base)
                            nc.tensor.matmul(
                                ps[:], lhsT=offw[:, ct, t, :], rhs=mov,
                                start=(mm == 0), stop=(mm == 17))
                            mm += 1
                    nc.scalar.activation(off_sb[0:18, rc * 400:(rc + 1) * 400], ps[:],
                                         ACT_IDENT, bias=offb[:])

                # index pipeline in two chunk-range stages so early-block
                # gathers start while the rest of the prologue still runs
                def index_stage(c0, c1):
                    nch = c1 - c0
                    ne = nch * 9
                    # transpose offsets via DMA xbar: [18, nch*128] -> [128, nch, 18]
                    nc.scalar.dma_start_transpose(
                        offT[:, c0:c1, :],
                        off_sb[:, c0 * 128:c1 * 128])

                    dy = revec(offT[:], [(NCHUNK * 32, 128), (32, nch), (2, 9)],
                               c0 * 32)
                    dx = revec(offT[:], [(NCHUNK * 32, 128), (32, nch), (2, 9)],
                               c0 * 32 + 1)
                    cybv = cyb[:, c0:c1, :]
                    cxbv = cxb[:, c0:c1, :]
                    pyb = ppool.tile([128, ne], fp32, tag="t_pyb")
                    pxb = ppool.tile([128, ne], fp32, tag="t_pxb")
                    t_i32 = ppool.tile([128, ne], i32, tag="t_ti")
                    y0f = ppool.tile([128, ne], fp32, tag="t_y0")
                    x0f = ppool.tile([128, ne], fp32, tag="t_x0")
                    fy = ppool.tile([128, ne], fp32, tag="t_fy")
                    fx = ppool.tile([128, ne], fp32, tag="t_fx")
                    gy = ppool.tile([128, ne], fp32, tag="t_gy")
                    gx = ppool.tile([128, ne], fp32, tag="t_gx")
                    gtt = ppool.tile([128, ne], fp32, tag="t_gt")
                    hi16 = ppool.tile([128, ne], fp16, tag="t_hi")
                    lo16 = ppool.tile([128, ne], fp16, tag="t_lo")

                    v3 = lambda t: revec(t[:], [(ne, 128), (9, nch), (1, 9)])
                    nc.vector.tensor_tensor(out=pyb[:], in0=dy, in1=cybv, op=A.add)
                    nc.vector.tensor_tensor(out=pxb[:], in0=dx, in1=cxbv, op=A.add)
                    nc.vector.tensor_scalar(pyb[:], pyb[:], CLIP_LO, CLIP_HI,
                                            A.max, A.min)
                    nc.vector.tensor_scalar(pxb[:], pxb[:], CLIP_LO, CLIP_HI,
                                            A.max, A.min)
                    # robust floor (works under trunc or round-to-nearest cvt):
                    # y0 = cvt(pyb); y0 -= (y0 > pyb)
                    nc.vector.tensor_copy(t_i32[:], pyb[:])
                    nc.vector.tensor_copy(y0f[:], t_i32[:])
                    nc.vector.tensor_tensor(out=gtt[:], in0=y0f[:], in1=pyb[:],
                                            op=A.is_gt)
                    nc.vector.tensor_tensor(out=y0f[:], in0=y0f[:], in1=gtt[:],
                                            op=A.subtract)
                    nc.vector.tensor_copy(t_i32[:], pxb[:])
                    nc.vector.tensor_copy(x0f[:], t_i32[:])
                    nc.vector.tensor_tensor(out=gtt[:], in0=x0f[:], in1=pxb[:],
                                            op=A.is_gt)
                    nc.vector.tensor_tensor(out=x0f[:], in0=x0f[:], in1=gtt[:],
                                            op=A.subtract)
                    nc.vector.tensor_tensor(out=fy[:], in0=pyb[:], in1=y0f[:],
                                            op=A.subtract)
                    nc.vector.tensor_tensor(out=fx[:], in0=pxb[:], in1=x0f[:],
                                            op=A.subtract)
                    # flat = (y0b*84 + x0b) - 1190, via stt then scalar add
                    nc.vector.scalar_tensor_tensor(pyb[:], y0f[:], float(PITCH),
                                                   x0f[:], A.mult, A.add)
                    nc.vector.tensor_scalar_add(pyb[:], pyb[:], float(FLAT_OFF))
                    # gy = 1-fy, gx = 1-fx
                    nc.vector.tensor_scalar(gy[:], fy[:], -1.0, 1.0, A.mult, A.add)
                    nc.vector.tensor_scalar(gx[:], fx[:], -1.0, 1.0, A.mult, A.add)
                    # beta[j]: b0=gx*gy b1=gx*fy b2=fx*gy b3=fx*fy
                    bj = lambda j: revec(beta[:],
                                         [(NE * 4, 128), (36, nch), (4, 9)],
                                         j + c0 * 36)
                    nc.vector.tensor_tensor(out=bj(0), in0=v3(gx), in1=v3(gy),
                                            op=A.mult)
                    nc.vector.tensor_tensor(out=bj(1), in0=v3(gx), in1=v3(fy),
                                            op=A.mult)
                    nc.vector.tensor_tensor(out=bj(2), in0=v3(fx), in1=v3(gy),
                                            op=A.mult)
                    nc.vector.tensor_tensor(out=bj(3), in0=v3(fx), in1=v3(fy),
                                            op=A.mult)

                    # idx fold via PE: flat (fp32 in pyb, values [0, 7056))
                    # split flat = 64*hi + lo (both fp16-exact), then per
                    # source group g two accumulating matmuls with selection
                    # matrices (entries 64.0 / 1.0) produce, replicated on
                    # every 16-partition stripe, out[16g'+q, (c,k)] =
                    # flat[16g+q, c, k]; strided casts land idxw[:, k, 8c+g].
                    # robust floor for hi (cvt may trunc or round-to-nearest)
                    nc.vector.tensor_scalar_mul(x0f[:], pyb[:], 1.0 / 64.0)
                    nc.vector.tensor_copy(t_i32[:], x0f[:])
                    nc.vector.tensor_copy(y0f[:], t_i32[:])
                    nc.vector.tensor_tensor(out=gtt[:], in0=y0f[:], in1=x0f[:],
                                            op=A.is_gt)
                    nc.vector.tensor_tensor(out=y0f[:], in0=y0f[:], in1=gtt[:],
                                            op=A.subtract)
                    nc.vector.tensor_copy(hi16[:], y0f[:])
                    nc.vector.scalar_tensor_tensor(x0f[:], y0f[:], -64.0,
                                                   pyb[:], A.mult, A.add)
                    nc.vector.tensor_copy(lo16[:], x0f[:])
                    for g in range(8):
                        fps = ppsum.tile([128, ne], fp32, tag="foldps", bufs=2)
                        nc.tensor.matmul(fps[:], lhsT=sel[:, g, 0, :],
                                         rhs=hi16[:], start=True, stop=False)
                        nc.tensor.matmul(fps[:], lhsT=sel[:, g, 1, :],
                                         rhs=lo16[:], start=False, stop=True)
                        f32 = ppool.tile([128, ne], i32, tag=f"t_f32_{g % 2}")
                        nc.vector.tensor_copy(f32[:], fps[:])
                        src = revec(f32[:], [(ne, 128), (1, 9), (9, nch)])
                        dst = revec(idxw[:], [(9 * (NPOS // 16), 128),
                                              (NPOS // 16, 9), (8, nch)],
                                    extra_offset=c0 * 8 + g)
                        nc.vector.tensor_copy(dst, src)

                NE = NCHUNK * 9   # 225
                # 4-stage pipeline: each pair of conv chunks (800 positions)
                # unlocks the next 6-7 index chunks, so the first gathers
                # launch while most of the offset conv is still running
                conv_chunk(0)
                conv_chunk(1)
                index_stage(0, 6)
                conv_chunk(2)
                conv_chunk(3)
                index_stage(6, 12)
                conv_chunk(4)
                conv_chunk(5)
                index_stage(12, 18)
                conv_chunk(6)
                conv_chunk(7)
                index_stage(18, NCHUNK)

            # ---------------- main loop ----------------
            with ExitStack() as mctx:
                mpool = mctx.enter_context(tc.tile_pool(name="main", bufs=1))
                mpsum = mctx.enter_context(
                    tc.tile_pool(name="mpsum", bufs=1, space="PSUM"))

                for gidx, (c0g, ng) in enumerate(GROUPS):
                    npos = ng * 128
                    base = c0g * 128
                    # valbuf/dt4 tags are created before the gb tags so the
                    # gather buffers land ABOVE the prologue-scratch address
                    # range: otherwise the first gathers WAR-wait on the last
                    # index-stage's reads (~60us stall)
                    valbuf = mpool.tile([128, 18, 512], fp16, tag="valbuf", bufs=2)

                    # batched diag-matrix build for the A-path taps:
                    # dt4[i][p, kk, j, c] = id16[p, c] * beta[p, c0g+i, kk, j]
                    dt4s = []
                    for i in range(ng):
                        ch = c0g + i
                        dt4 = mpool.tile([128, N_DIAG, 4, 128], fp16,
                                         tag="dt4", bufs=4)
                        in0 = revec(id16[:],
                                    [(128, 128), (0, N_DIAG * 4), (1, 128)])
                        in1 = revec(beta[:],
                                    [(NCHUNK * 36, 128), (1, N_DIAG * 4),
                                     (0, 128)], ch * 36)
                        nc.vector.tensor_tensor(out=dt4[:], in0=in0, in1=in1,
                                                op=A.mult)
                        dt4s.append(dt4)

                    gbs = []
                    for k in range(9):
                        gb = mpool.tile([128, 4, 1024], fp16, tag=f"gb{k}",
                                        bufs=(2 if k >= 4 else 1))
                        nc.gpsimd.dma_gather(
                            gb[:, :ng, :], x2_view,
                            idxw[:, k, c0g * 8:(c0g + ng) * 8],
                            npos, npos, 1024, elem_step=512,
                            queue_num=k % 4)
                        gbs.append(gb)

                    for k in range(9):
                        if k < N_DIAG:
                            # PE diag-matmul path (scale+sum+transpose on PE)
                            pv0 = mpsum.tile([128, 512], fp32, tag="pva0",
                                             bufs=2)
                            pv1 = mpsum.tile([128, 512], fp32, tag="pva1",
                                             bufs=2)
                            for i in range(ng):
                                for ct, pv in ((0, pv0), (1, pv1)):
                                    for j in range(4):
                                        slot = (j // 2) * 4 + (j % 2) * 2 + ct
                                        nc.tensor.matmul(
                                            pv[:, i * 128:(i + 1) * 128],
                                            lhsT=gbs[k][:, i,
                                                        slot * 128:(slot + 1) * 128],
                                            rhs=dt4s[i][:, k, j, :],
                                            start=(j == 0), stop=(j == 3))
                        else:
                            # scale+sum path (scalar j0/j1, DVE j2/j3 fused)
                            # with plain PE transposes
                            pv0 = mpsum.tile([128, 512], fp16, tag="pvf0",
                                             bufs=2)
                            pv1 = mpsum.tile([128, 512], fp16, tag="pvf1",
                                             bufs=2)
                            for i in range(ng):
                                ch = c0g + i
                                scr = mpool.tile([128, 4, 256], fp16,
                                                 tag="scr", bufs=4)
                                s2 = mpool.tile([128, 2, 256], fp16,
                                                tag="s2", bufs=4)
                                summ = mpool.tile([128, 256], fp16,
                                                  tag="summ", bufs=4)
                                for j in range(2):
                                    nc.scalar.activation(
                                        scr[:, j, :],
                                        gbs[k][:, i, j * 256:(j + 1) * 256],
                                        ACT_COPY,
                                        scale=beta[:, ch, k, j:j + 1])
                                bb = revec(beta[:],
                                           [(NCHUNK * 36, 128), (1, 2),
                                            (0, 256)], ch * 36 + k * 4 + 2)
                                nc.vector.tensor_tensor(
                                    out=scr[:, 2:4, :],
                                    in0=gbs[k][:, i, 512:1024],
                                    in1=bb, op=A.mult)
                                nc.vector.tensor_tensor(
                                    out=s2[:], in0=scr[:, 0:2, :],
                                    in1=scr[:, 2:4, :], op=A.add)
                                nc.vector.tensor_tensor(
                                    out=summ[:], in0=s2[:, 0, :],
                                    in1=s2[:, 1, :], op=A.add)
                                nc.tensor.matmul(
                                    pv0[:, i * 128:(i + 1) * 128],
                                    lhsT=summ[:, 0:128],
                                    rhs=id16[:], start=True, stop=True,
                                    is_transpose=True)
                                nc.tensor.matmul(
                                    pv1[:, i * 128:(i + 1) * 128],
                                    lhsT=summ[:, 128:256],
                                    rhs=id16[:], start=True, stop=True,
                                    is_transpose=True)
                        nc.scalar.activation(valbuf[:, k, :npos],
                                             pv0[:, :npos], ACT_COPY)
                        nc.scalar.activation(valbuf[:, 9 + k, :npos],
                                             pv1[:, :npos], ACT_COPY)

                    for ot in range(2):
                        po = mpsum.tile([128, 512], fp32, tag=f"po{ot}", bufs=1)
                        for ci in range(18):
                            ct, k = ci // 9, ci % 9
                            nc.tensor.matmul(
                                po[:, :npos],
                                lhsT=convw[:, ct, k, ot * 128:(ot + 1) * 128],
                                rhs=valbuf[:, ci, :npos],
                                start=(ci == 0), stop=(ci == 17))
                        osb = mpool.tile([128, 512], fp16, tag="osb", bufs=2)
                        nc.scalar.activation(osb[:, :npos], po[:, :npos],
                                             ACT_IDENT, bias=convb[:, ot:ot + 1])
                        nc.sync.dma_start(
                            out=out_d[ot, :, base:base + npos],
                            in_=osb[:, :npos])

    nc.compile()
    return nc


def _host_prep(x, offset_w, offset_b, conv_w, conv_b):
    """Build per-core input maps."""
    x = np.asarray(x, np.float32)
    offset_w = np.asarray(offset_w, np.float32)
    offset_b = np.asarray(offset_b, np.float32)
    conv_w = np.asarray(conv_w, np.float32)
    conv_b = np.asarray(conv_b, np.float32)

    # weights, shared
    # offset_w: [18, 256, 3, 3] -> [c128, ct, t, d]
    ow = offset_w.reshape(18, 2, 128, 3, 3)
    offw_h = np.ascontiguousarray(
        ow.reshape(18, 2, 128, 9).transpose(2, 1, 3, 0)).astype(np.float16)
    offb_h = offset_b.reshape(18, 1).astype(np.float32)
    cw = conv_w.reshape(256, 2, 128, 9)
    convw_h = np.ascontiguousarray(cw.transpose(2, 1, 3, 0)).astype(np.float16)  # [c,ct,t,o]
    convb_h = np.ascontiguousarray(
        conv_b.reshape(2, 128).transpose(1, 0)).astype(np.float32)  # [c128, ot]
    id16_h = np.eye(128, dtype=np.float16)
    # fold selection matrices: sel[r, g, 0/1, m] = 64/1 at m%16 == r%16 for
    # source group g == r//16, replicated over all 8 dest 16-part stripes
    sel_h = np.zeros((128, 8, 2, 128), np.float16)
    r = np.arange(128)
    m = np.arange(128)
    hit = (m[None, :] % 16) == (r[:, None] % 16)      # [r, m]
    for g in range(8):
        gm = hit & ((r[:, None] // 16) == g)
        sel_h[:, g, 0, :] = gm * np.float16(64.0)
        sel_h[:, g, 1, :] = gm * np.float16(1.0)

    # per-core base constants
    k = np.arange(9)
    ry = (k // 3 - 1).astype(np.float32)
    rx = (k % 3 - 1).astype(np.float32)
    in_maps = []
    per_sample = {}
    for b in range(B):
        xc = np.ascontiguousarray(x[b].transpose(1, 2, 0))       # [H, W, C]
        xp = np.pad(xc, ((2, 2), (2, 2), (0, 0))).astype(np.float16)  # [84, 84, 256]
        x2 = np.zeros((PITCH, PITCH, 2, 256), np.float16)
        x2[:83, :, 0] = xp[:83]
        x2[:83, :, 1] = xp[1:84]
        x2_h = x2.reshape(NGROUPS, 512)
        per_sample[b] = (x2_h, xp)

    for core in range(NCORES):
        b, half = core // 2, core % 2
        h0 = half * HHALF
        x2_h, xp = per_sample[b]
        # xcf: channel-first, rows [h0-2 .. h0+42) of the padded image
        # relocated to local rows [0..44)
        xcf_rows = xp[h0:h0 + 44]                                # [44, 84, 256]
        xcf_h = np.ascontiguousarray(
            xcf_rows.transpose(2, 0, 1).reshape(2, 128, 44 * PITCH)
            .transpose(1, 0, 2))

        i = np.arange(NPOS)
        hloc = i // W
        wloc = i % W
        cyb_h = ((h0 + hloc)[:, None] + ry[None, :] + FBIAS).astype(np.float32)
        cxb_h = (wloc[:, None] + rx[None, :] + FBIAS).astype(np.float32)
        cyb_h = np.ascontiguousarray(
            cyb_h.reshape(NCHUNK, 128, 9).transpose(1, 0, 2))
        cxb_h = np.ascontiguousarray(
            cxb_h.reshape(NCHUNK, 128, 9).transpose(1, 0, 2))

        in_maps.append({
            "x2": x2_h, "xcf": xcf_h, "offw": offw_h, "offb": offb_h,
            "convw": convw_h, "convb": convb_h, "cyb": cyb_h, "cxb": cxb_h,
            "id16": id16_h, "sel": sel_h,
        })
    return in_maps


def kernel(x, offset_w, offset_b, conv_w, conv_b, _trace=False):
    from concourse.bass_utils import run_bass_kernel_spmd

    if "nc" not in _cached:
        _cached["nc"] = _build_program()
    nc = _cached["nc"]
    in_maps = _host_prep(x, offset_w, offset_b, conv_w, conv_b)
    res = run_bass_kernel_spmd(nc, in_maps, list(range(NCORES)), trace=_trace)
    _cached["last_result"] = res
    out = np.zeros((B, COUT, H, W), np.float32)
    for core in range(NCORES):
        b, half = core // 2, core % 2
        o = res.results[core]["out"].astype(np.float32)   # [2, 128, NPOS]
        out[b, :, half * HHALF:(half + 1) * HHALF, :] = \
            o.reshape(COUT, HHALF, W)
    return out



# revision 45
# speedup vs baseline: 1.3216x; 1.0424x over previous
"""Deformable conv2d (3x3, pad 1) on 8 trn2 NeuronCores.

Sharding: (batch b, image half) -> core 2*b + half. Each core:
  0. PE p-state warmup (ldweights-free identity transposes) so the serial
     offset conv runs at full clock from its first matmul
  1. offset conv (PE matmuls over 2 c-tiles x 9 taps, N=400 position chunks)
  2. DMA-xbar transpose offsets to position-major; DVE index/bilinear math
     staged over two chunk ranges so early gathers overlap the prologue
  3. dma_gather of 2x2 fp16 pixel patches (all 256 ch) from an interleaved
     row-pair HBM image, positions-on-partitions
  4. bilinear combine, hybrid per tap: taps < N_DIAG run diag-matmul
     accumulation on PE (rhs = batched broadcast-built scaled identities);
     the rest run per-partition beta scales + corner sums on DVE (fp16 4x)
     with plain PE transposes (fixed identity stationary)
  5. main conv = 18-chunk PE accumulation over (c-tile, tap); conv bias is
     applied by the output activation copy (per-partition bias), fp16 store.
"""
import numpy as np

B, CIN, COUT, H, W = 4, 256, 256, 80, 80
NCORES = 8
HHALF = H // 2                      # 40 rows per core
NPOS = HHALF * W                    # 3200 positions per core
NCHUNK = NPOS // 128                # 25
PITCH = 84                          # x2 pixel-group pitch per row
NGROUPS = PITCH * PITCH             # 7056 (rows in x2; 83*84 + slack)
FBIAS = 16.0                        # float->int truncation bias (floor trick)
CLIP_LO = 14.0                      # = -2 + FBIAS
CLIP_HI = 96.9                      # = 80.9 + FBIAS
FLAT_OFF = -(14 * PITCH + 14)       # flat = y0b*84 + x0b + FLAT_OFF
STAGES = [(0, 6), (6, 12), (12, 18), (18, 25)]   # index-stage chunk ranges
GROUPS = [(0, 3), (3, 3), (6, 3), (9, 3), (12, 3), (15, 3), (18, 3), (21, 4)]
N_DIAG = 9          # all taps use the PE diag-matmul bilinear path

_cached = {}


def _build_program():
    from concourse import bass, bacc, tile, mybir
    from contextlib import ExitStack

    fp16, fp32 = mybir.dt.float16, mybir.dt.float32
    i16, i32 = mybir.dt.int16, mybir.dt.int32
    A = mybir.AluOpType
    ACT_COPY = mybir.ActivationFunctionType.Copy
    ACT_IDENT = mybir.ActivationFunctionType.Identity

    nc = bacc.Bacc("TRN2", target_bir_lowering=False, debug=False,
                   num_devices=NCORES, num_swdge_queues=4)

    x2_d = nc.dram_tensor("x2", [NGROUPS, 512], fp16, kind="ExternalInput")
    xcf_d = nc.dram_tensor("xcf", [128, 2, 44 * PITCH], fp16, kind="ExternalInput")
    offw_d = nc.dram_tensor("offw", [128, 2, 9, 18], fp16, kind="ExternalInput")
    offb_d = nc.dram_tensor("offb", [18, 1], fp32, kind="ExternalInput")
    convw_d = nc.dram_tensor("convw", [128, 2, 9, 256], fp16, kind="ExternalInput")
    convb_d = nc.dram_tensor("convb", [128, 2], fp32, kind="ExternalInput")
    cyb_d = nc.dram_tensor("cyb", [128, NCHUNK, 9], fp32, kind="ExternalInput")
    cxb_d = nc.dram_tensor("cxb", [128, NCHUNK, 9], fp32, kind="ExternalInput")
    id16_d = nc.dram_tensor("id16", [128, 128], fp16, kind="ExternalInput")
    sel_d = nc.dram_tensor("sel", [128, 8, 2, 128], fp16, kind="ExternalInput")
    out_d = nc.dram_tensor("out", [2, 128, NPOS], fp16, kind="ExternalOutput")

    # overlapping gather-source view: [NGROUPS-1, 1024] with row stride 512
    x2_view = x2_d.ap().copy()
    v = x2_view.ap
    v[0] = [512, NGROUPS - 1]
    v[1] = [1, 1024]
    x2_view.ap = v

    def revec(ap, dims, extra_offset=0):
        """Rebuild an AP's dim list: dims = [(stride, num), ...]."""
        a = ap.copy()
        vv = a.ap
        while len(vv) > 1:
            vv.pop()
        vv[0] = list(dims[0])
        for d in dims[1:]:
            vv.append(list(d))
        a.ap = vv
        a.offset = a.offset + extra_offset
        return a

    with tile.TileContext(nc) as tc:
        with ExitStack() as ctx:
            persist = ctx.enter_context(tc.tile_pool(name="persist", bufs=1))
            # per-stage idx tiles: keeps each fold's write extent inside one
            # tile so early groups' gathers don't conservatively depend on
            # later index stages
            idxws = [persist.tile([128, 9, 8 * (c1 - c0)], i16,
                                  name=f"idxw{si}")
                     for si, (c0, c1) in enumerate(STAGES)]
            beta = persist.tile([128, NCHUNK, 9, 4], fp16)
            id16 = persist.tile([128, 128], fp16)
            convw = persist.tile([128, 2, 9, 256], fp16)
            convb = persist.tile([128, 2], fp32)

            # ---------------- prologue: offsets + indices ----------------
            with ExitStack() as pctx:
                ppool = pctx.enter_context(tc.tile_pool(name="pro", bufs=1))
                ppsum = pctx.enter_context(
                    tc.tile_pool(name="ppsum", bufs=2, space="PSUM"))

                xcf = ppool.tile([128, 2, 44 * PITCH], fp16)
                cyb = ppool.tile([128, NCHUNK, 9], fp32)
                cxb = ppool.tile([128, NCHUNK, 9], fp32)
                off_sb = ppool.tile([32, NPOS], fp16)
                offT = ppool.tile([128, NCHUNK, 32], fp16)
                nc.vector.memset(off_sb[:], 0.0)
                offw = ppool.tile([128, 2, 9, 18], fp16)
                offb = ppool.tile([18, 1], fp32)
                sel = ppool.tile([128, 8, 2, 128], fp16)
                nc.sync.dma_start(out=sel[:], in_=sel_d[:])
                nc.sync.dma_start(out=id16[:], in_=id16_d[:])
                nc.sync.dma_start(out=offw[:], in_=offw_d[:])
                nc.sync.dma_start(out=offb[:], in_=offb_d[:])
                SPL1, SPL2 = 8 * PITCH, 20 * PITCH
                nc.sync.dma_start(out=xcf[:, :, :SPL1], in_=xcf_d[:, :, :SPL1])
                nc.sync.dma_start(out=xcf[:, :, SPL1:SPL2],
                                  in_=xcf_d[:, :, SPL1:SPL2])
                nc.sync.dma_start(out=xcf[:, :, SPL2:], in_=xcf_d[:, :, SPL2:])
                nc.sync.dma_start(out=cyb[:], in_=cyb_d[:])
                nc.sync.dma_start(out=cxb[:], in_=cxb_d[:])
                nc.sync.dma_start(out=convw[:], in_=convw_d[:])
                nc.sync.dma_start(out=convb[:], in_=convb_d[:])

                # PE p-state warmup: ldweights-free transposes keep the PE
                # continuously busy from t~0 so the offset conv starts at
                # full clock instead of ramping from the low p-state.
                wps = ppsum.tile([128, 128], fp16, tag="warm", bufs=1)
                for _ in range(45):
                    nc.tensor.matmul(wps[:], lhsT=id16[:], rhs=id16[:],
                                     start=True, stop=True, is_transpose=True)

                # offset conv: 8 chunks x 5 rows x 80 cols (N=400).
                # xcf holds the core's 44-row padded window relocated to rows
                # [0..44); conv output row r (0..39) reads xcf rows (r + ky).
                def conv_chunk(rc):
                    ps = ppsum.tile([18, 400], fp32, tag="offps")
                    mm = 0
                    for ct in range(2):
                        for t in range(9):
                            ky, kx = t // 3, t % 3
                            base = (rc * 5 + ky + 1) * PITCH + kx + 1
                            mov = revec(xcf[:, ct, 0],
                                        [(2 * 44 * PITCH, 128), (PITCH, 5), (1, 80)],
                                        extra_offset=base)
                            nc.tensor.matmul(
                                ps[:], lhsT=offw[:, ct, t, :], rhs=mov,
                                start=(mm == 0), stop=(mm == 17))
                            mm += 1
                    nc.scalar.activation(off_sb[0:18, rc * 400:(rc + 1) * 400], ps[:],
                                         ACT_IDENT, bias=offb[:])

                # index pipeline in two chunk-range stages so early-block
                # gathers start while the rest of the prologue still runs
                def index_stage(si):
                    c0, c1 = STAGES[si]
                    nch = c1 - c0
                    ne = nch * 9
                    # transpose offsets via DMA xbar: [18, nch*128] -> [128, nch, 18]
                    nc.scalar.dma_start_transpose(
                        offT[:, c0:c1, :],
                        off_sb[:, c0 * 128:c1 * 128])

                    dy = revec(offT[:], [(NCHUNK * 32, 128), (32, nch), (2, 9)],
                               c0 * 32)
                    dx = revec(offT[:], [(NCHUNK * 32, 128), (32, nch), (2, 9)],
                               c0 * 32 + 1)
                    cybv = cyb[:, c0:c1, :]
                    cxbv = cxb[:, c0:c1, :]
                    pyb = ppool.tile([128, ne], fp32, tag="t_pyb")
                    pxb = ppool.tile([128, ne], fp32, tag="t_pxb")
                    t_i32 = ppool.tile([128, ne], i32, tag="t_ti")
                    y0f = ppool.tile([128, ne], fp32, tag="t_y0")
                    x0f = ppool.tile([128, ne], fp32, tag="t_x0")
                    fy = ppool.tile([128, ne], fp32, tag="t_fy")
                    fx = ppool.tile([128, ne], fp32, tag="t_fx")
                    gy = ppool.tile([128, ne], fp32, tag="t_gy")
                    gx = ppool.tile([128, ne], fp32, tag="t_gx")
                    gtt = ppool.tile([128, ne], fp32, tag="t_gt")
                    hi16 = ppool.tile([128, ne], fp16, tag="t_hi")
                    lo16 = ppool.tile([128, ne], fp16, tag="t_lo")

                    v3 = lambda t: revec(t[:], [(ne, 128), (9, nch), (1, 9)])
                    nc.vector.tensor_tensor(out=pyb[:], in0=dy, in1=cybv, op=A.add)
                    nc.vector.tensor_tensor(out=pxb[:], in0=dx, in1=cxbv, op=A.add)
                    nc.vector.tensor_scalar(pyb[:], pyb[:], CLIP_LO, CLIP_HI,
                                            A.max, A.min)
                    nc.vector.tensor_scalar(pxb[:], pxb[:], CLIP_LO, CLIP_HI,
                                            A.max, A.min)
                    # robust floor (works under trunc or round-to-nearest cvt):
                    # y0 = cvt(pyb); y0 -= (y0 > pyb)
                    nc.vector.tensor_copy(t_i32[:], pyb[:])
                    nc.vector.tensor_copy(y0f[:], t_i32[:])
                    nc.vector.tensor_tensor(out=gtt[:], in0=y0f[:], in1=pyb[:],
                                            op=A.is_gt)
                    nc.vector.tensor_tensor(out=y0f[:], in0=y0f[:], in1=gtt[:],
                                            op=A.subtract)
                    nc.vector.tensor_copy(t_i32[:], pxb[:])
                    nc.vector.tensor_copy(x0f[:], t_i32[:])
                    nc.vector.tensor_tensor(out=gtt[:], in0=x0f[:], in1=pxb[:],
                                            op=A.is_gt)
                    nc.vector.tensor_tensor(out=x0f[:], in0=x0f[:], in1=gtt[:],
                                            op=A.subtract)
                    nc.vector.tensor_tensor(out=fy[:], in0=pyb[:], in1=y0f[:],
                                            op=A.subtract)
                    nc.vector.tensor_tensor(out=fx[:], in0=pxb[:], in1=x0f[:],
                                            op=A.subtract)
                    # flat = (y0b*84 + x0b) - 1190, via stt then scalar add
                    nc.vector.scalar_tensor_tensor(pyb[:], y0f[:], float(PITCH),
                                                   x0f[:], A.mult, A.add)
                    nc.vector.tensor_scalar_add(pyb[:], pyb[:], float(FLAT_OFF))
                    # gy = 1-fy, gx = 1-fx
                    nc.vector.tensor_scalar(gy[:], fy[:], -1.0, 1.0, A.mult, A.add)
                    nc.vector.tensor_scalar(gx[:], fx[:], -1.0, 1.0, A.mult, A.add)
                    # beta[j]: b0=gx*gy b1=gx*fy b2=fx*gy b3=fx*fy — products
                    # land contiguous, then one strided cast-copy into beta
                    prod = ppool.tile([128, 4, ne], fp32, tag="t_prod")
                    nc.vector.tensor_tensor(out=prod[:, 0, :], in0=gx[:],
                                            in1=gy[:], op=A.mult)
                    nc.vector.tensor_tensor(out=prod[:, 1, :], in0=gx[:],
                                            in1=fy[:], op=A.mult)
                    nc.vector.tensor_tensor(out=prod[:, 2, :], in0=fx[:],
                                            in1=gy[:], op=A.mult)
                    nc.vector.tensor_tensor(out=prod[:, 3, :], in0=fx[:],
                                            in1=fy[:], op=A.mult)
                    bdst = revec(beta[:], [(NCHUNK * 36, 128), (1, 4),
                                           (36, nch), (4, 9)], c0 * 36)
                    bsrc = revec(prod[:], [(4 * ne, 128), (ne, 4),
                                           (9, nch), (1, 9)])
                    nc.vector.tensor_copy(bdst, bsrc)

                    # idx fold via PE: flat (fp32 in pyb, values [0, 7056))
                    # split flat = 64*hi + lo (both fp16-exact), then per
                    # source group g two accumulating matmuls with selection
                    # matrices (entries 64.0 / 1.0) produce, replicated on
                    # every 16-partition stripe, out[16g'+q, (c,k)] =
                    # flat[16g+q, c, k]; strided casts land idxw[:, k, 8c+g].
                    # robust floor for hi (cvt may trunc or round-to-nearest)
                    nc.vector.tensor_scalar_mul(x0f[:], pyb[:], 1.0 / 64.0)
                    nc.vector.tensor_copy(t_i32[:], x0f[:])
                    nc.vector.tensor_copy(y0f[:], t_i32[:])
                    nc.vector.tensor_tensor(out=gtt[:], in0=y0f[:], in1=x0f[:],
                                            op=A.is_gt)
                    nc.vector.tensor_tensor(out=y0f[:], in0=y0f[:], in1=gtt[:],
                                            op=A.subtract)
                    nc.vector.tensor_copy(hi16[:], y0f[:])
                    nc.vector.scalar_tensor_tensor(x0f[:], y0f[:], -64.0,
                                                   pyb[:], A.mult, A.add)
                    nc.vector.tensor_copy(lo16[:], x0f[:])
                    for g in range(8):
                        fps = ppsum.tile([128, ne], fp32, tag="foldps", bufs=2)
                        nc.tensor.matmul(fps[:], lhsT=sel[:, g, 0, :],
                                         rhs=hi16[:], start=True, stop=False)
                        nc.tensor.matmul(fps[:], lhsT=sel[:, g, 1, :],
                                         rhs=lo16[:], start=False, stop=True)
                        f32 = ppool.tile([128, ne], i32, tag=f"t_f32_{g % 2}")
                        nc.vector.tensor_copy(f32[:], fps[:])
                        src = revec(f32[:], [(ne, 128), (1, 9), (9, nch)])
                        dst = revec(idxws[si][:], [(9 * 8 * nch, 128),
                                                   (8 * nch, 9), (8, nch)],
                                    extra_offset=g)
                        nc.vector.tensor_copy(dst, src)

                # 4-stage pipeline: each pair of conv chunks (800 positions)
                # unlocks the next 6-7 index chunks, so the first gathers
                # launch while most of the offset conv is still running
                conv_chunk(0)
                conv_chunk(1)
                index_stage(0)
                conv_chunk(2)
                conv_chunk(3)
                index_stage(1)
                conv_chunk(4)
                conv_chunk(5)
                index_stage(2)
                conv_chunk(6)
                conv_chunk(7)
                index_stage(3)

            # ---------------- main loop ----------------
            with ExitStack() as mctx:
                mpool = mctx.enter_context(tc.tile_pool(name="main", bufs=1))
                mpsum = mctx.enter_context(
                    tc.tile_pool(name="mpsum", bufs=1, space="PSUM"))

                for gidx, (c0g, ng) in enumerate(GROUPS):
                    npos = ng * 128
                    base = c0g * 128
                    # valbuf/dt4 tags are created before the gb tags so the
                    # gather buffers land ABOVE the prologue-scratch address
                    # range: otherwise the first gathers WAR-wait on the last
                    # index-stage's reads (~60us stall)
                    valbuf = mpool.tile([128, 18, 512], fp16, tag="valbuf", bufs=2)

                    # batched diag-matrix build for the A-path taps:
                    # dt4[i][p, kk, j, c] = id16[p, c] * beta[p, c0g+i, kk, j]
                    dt4s = []
                    for i in range(ng):
                        ch = c0g + i
                        dt4 = mpool.tile([128, N_DIAG, 4, 128], fp16,
                                         tag="dt4", bufs=4)
                        in0 = revec(id16[:],
                                    [(128, 128), (0, N_DIAG * 4), (1, 128)])
                        in1 = revec(beta[:],
                                    [(NCHUNK * 36, 128), (1, N_DIAG * 4),
                                     (0, 128)], ch * 36)
                        nc.vector.tensor_tensor(out=dt4[:], in0=in0, in1=in1,
                                                op=A.mult)
                        dt4s.append(dt4)

                    si = next(i for i, (a, b) in enumerate(STAGES)
                              if a <= c0g and c0g + ng <= b)
                    st0 = STAGES[si][0]
                    gbs = []
                    for k in range(9):
                        gb = mpool.tile([128, 4, 1024], fp16, tag=f"gb{k}",
                                        bufs=(2 if k >= 4 else 1))
                        nc.gpsimd.dma_gather(
                            gb[:, :ng, :], x2_view,
                            idxws[si][:, k,
                                      (c0g - st0) * 8:(c0g - st0 + ng) * 8],
                            npos, npos, 1024, elem_step=512,
                            queue_num=k % 4)
                        gbs.append(gb)

                    for k in range(9):
                        if k < N_DIAG:
                            # PE diag-matmul path (scale+sum+transpose on PE)
                            pv0 = mpsum.tile([128, 512], fp32, tag="pva0",
                                             bufs=2)
                            pv1 = mpsum.tile([128, 512], fp32, tag="pva1",
                                             bufs=2)
                            for i in range(ng):
                                for ct, pv in ((0, pv0), (1, pv1)):
                                    for j in range(4):
                                        slot = (j // 2) * 4 + (j % 2) * 2 + ct
                                        nc.tensor.matmul(
                                            pv[:, i * 128:(i + 1) * 128],
                                            lhsT=gbs[k][:, i,
                                                        slot * 128:(slot + 1) * 128],
                                            rhs=dt4s[i][:, k, j, :],
                                            start=(j == 0), stop=(j == 3))
                        else:
                            # scale+sum path (scalar j0/j1, DVE j2/j3 fused)
                            # with plain PE transposes
                            pv0 = mpsum.tile([128, 512], fp16, tag="pvf0",
                                             bufs=2)
                            pv1 = mpsum.tile([128, 512], fp16, tag="pvf1",
                                             bufs=2)
                            for i in range(ng):
                                ch = c0g + i
                                scr = mpool.tile([128, 4, 256], fp16,
                                                 tag="scr", bufs=4)
                                s2 = mpool.tile([128, 2, 256], fp16,
                                                tag="s2", bufs=4)
                                summ = mpool.tile([128, 256], fp16,
                                                  tag="summ", bufs=4)
                                for j in range(2):
                                    nc.scalar.activation(
                                        scr[:, j, :],
                                        gbs[k][:, i, j * 256:(j + 1) * 256],
                                        ACT_COPY,
                                        scale=beta[:, ch, k, j:j + 1])
                                bb = revec(beta[:],
                                           [(NCHUNK * 36, 128), (1, 2),
                                            (0, 256)], ch * 36 + k * 4 + 2)
                                nc.vector.tensor_tensor(
                                    out=scr[:, 2:4, :],
                                    in0=gbs[k][:, i, 512:1024],
                                    in1=bb, op=A.mult)
                                nc.vector.tensor_tensor(
                                    out=s2[:], in0=scr[:, 0:2, :],
                                    in1=scr[:, 2:4, :], op=A.add)
                                nc.vector.tensor_tensor(
                                    out=summ[:], in0=s2[:, 0, :],
                                    in1=s2[:, 1, :], op=A.add)
                                nc.tensor.matmul(
                                    pv0[:, i * 128:(i + 1) * 128],
                                    lhsT=summ[:, 0:128],
                                    rhs=id16[:], start=True, stop=True,
                                    is_transpose=True)
                                nc.tensor.matmul(
                                    pv1[:, i * 128:(i + 1) * 128],
                                    lhsT=summ[:, 128:256],
                                    rhs=id16[:], start=True, stop=True,
                                    is_transpose=True)
                        nc.scalar.activation(valbuf[:, k, :npos],
                                             pv0[:, :npos], ACT_COPY)
                        nc.scalar.activation(valbuf[:, 9 + k, :npos],
                                             pv1[:, :npos], ACT_COPY)

                    for ot in range(2):
                        po = mpsum.tile([128, 512], fp32, tag=f"po{ot}", bufs=1)
                        for ci in range(18):
                            ct, k = ci // 9, ci % 9
                            nc.tensor.matmul(
                                po[:, :npos],
                                lhsT=convw[:, ct, k, ot * 128:(ot + 1) * 128],
                                rhs=valbuf[:, ci, :npos],
                                start=(ci == 0), stop=(ci == 17))
                        osb = mpool.tile([128, 512], fp16, tag="osb", bufs=2)
                        nc.scalar.activation(osb[:, :npos], po[:, :npos],
                                             ACT_IDENT, bias=convb[:, ot:ot + 1])
                        nc.sync.dma_start(
                            out=out_d[ot, :, base:base + npos],
                            in_=osb[:, :npos])

    nc.compile()
    return nc


def _host_prep(x, offset_w, offset_b, conv_w, conv_b):
    """Build per-core input maps."""
    x = np.asarray(x, np.float32)
    offset_w = np.asarray(offset_w, np.float32)
    offset_b = np.asarray(offset_b, np.float32)
    conv_w = np.asarray(conv_w, np.float32)
    conv_b = np.asarray(conv_b, np.float32)

    # weights, shared
    # offset_w: [18, 256, 3, 3] -> [c128, ct, t, d]
    ow = offset_w.reshape(18, 2, 128, 3, 3)
    offw_h = np.ascontiguousarray(
        ow.reshape(18, 2, 128, 9).transpose(2, 1, 3, 0)).astype(np.float16)
    offb_h = offset_b.reshape(18, 1).astype(np.float32)
    cw = conv_w.reshape(256, 2, 128, 9)
    convw_h = np.ascontiguousarray(cw.transpose(2, 1, 3, 0)).astype(np.float16)  # [c,ct,t,o]
    convb_h = np.ascontiguousarray(
        conv_b.reshape(2, 128).transpose(1, 0)).astype(np.float32)  # [c128, ot]
    id16_h = np.eye(128, dtype=np.float16)
    # fold selection matrices: sel[r, g, 0/1, m] = 64/1 at m%16 == r%16 for
    # source group g == r//16, replicated over all 8 dest 16-part stripes
    sel_h = np.zeros((128, 8, 2, 128), np.float16)
    r = np.arange(128)
    m = np.arange(128)
    hit = (m[None, :] % 16) == (r[:, None] % 16)      # [r, m]
    for g in range(8):
        gm = hit & ((r[:, None] // 16) == g)
        sel_h[:, g, 0, :] = gm * np.float16(64.0)
        sel_h[:, g, 1, :] = gm * np.float16(1.0)

    # per-core base constants
    k = np.arange(9)
    ry = (k // 3 - 1).astype(np.float32)
    rx = (k % 3 - 1).astype(np.float32)
    in_maps = []
    per_sample = {}
    for b in range(B):
        xc = np.ascontiguousarray(x[b].transpose(1, 2, 0))       # [H, W, C]
        xp = np.pad(xc, ((2, 2), (2, 2), (0, 0))).astype(np.float16)  # [84, 84, 256]
        x2 = np.zeros((PITCH, PITCH, 2, 256), np.float16)
        x2[:83, :, 0] = xp[:83]
        x2[:83, :, 1] = xp[1:84]
        x2_h = x2.reshape(NGROUPS, 512)
        per_sample[b] = (x2_h, xp)

    for core in range(NCORES):
        b, half = core // 2, core % 2
        h0 = half * HHALF
        x2_h, xp = per_sample[b]
        # xcf: channel-first, rows [h0-2 .. h0+42) of the padded image
        # relocated to local rows [0..44)
        xcf_rows = xp[h0:h0 + 44]                                # [44, 84, 256]
        xcf_h = np.ascontiguousarray(
            xcf_rows.transpose(2, 0, 1).reshape(2, 128, 44 * PITCH)
            .transpose(1, 0, 2))

        i = np.arange(NPOS)
        hloc = i // W
        wloc = i % W
        cyb_h = ((h0 + hloc)[:, None] + ry[None, :] + FBIAS).astype(np.float32)
        cxb_h = (wloc[:, None] + rx[None, :] + FBIAS).astype(np.float32)
        cyb_h = np.ascontiguousarray(
            cyb_h.reshape(NCHUNK, 128, 9).transpose(1, 0, 2))
        cxb_h = np.ascontiguousarray(
            cxb_h.reshape(NCHUNK, 128, 9).transpose(1, 0, 2))

        in_maps.append({
            "x2": x2_h, "xcf": xcf_h, "offw": offw_h, "offb": offb_h,
            "convw": convw_h, "convb": convb_h, "cyb": cyb_h, "cxb": cxb_h,
            "id16": id16_h, "sel": sel_h,
        })
    return in_maps


def kernel(x, offset_w, offset_b, conv_w, conv_b, _trace=False):
    from concourse.bass_utils import run_bass_kernel_spmd

    if "nc" not in _cached:
        _cached["nc"] = _build_program()
    nc = _cached["nc"]
    in_maps = _host_prep(x, offset_w, offset_b, conv_w, conv_b)
    res = run_bass_kernel_spmd(nc, in_maps, list(range(NCORES)), trace=_trace)
    _cached["last_result"] = res
    out = np.zeros((B, COUT, H, W), np.float32)
    for core in range(NCORES):
        b, half = core // 2, core % 2
        o = res.results[core]["out"].astype(np.float32)   # [2, 128, NPOS]
        out[b, :, half * HHALF:(half + 1) * HHALF, :] = \
            o.reshape(COUT, HHALF, W)
    return out

